# revision 17
# baseline (speedup 1.0000x reference)
"""Trainium2 kernel for nn_NodeMemory (scatter_memory GRU node-memory update).

Strategy
--------
The 512 MB memory table never touches the device. On the host we gather the
131072 referenced rows (memory[node_ids]), transpose the batch into
feature-major layout, and split the update batch evenly across the 8
NeuronCores. Each core runs a Bass/Tile kernel computing the GRU cell for its
16384 rows with the hidden/gate dimension on SBUF partitions:

    giT = W_ih @ x.T   (2 K-tiles of 128)      ghT = W_hh @ h.T   (1 K-tile)
    r = sigmoid(giT_r + ghT_r + b_r)           z = sigmoid(giT_z + ghT_z + b_z)
    n = tanh(giT_n + b_in + r * (ghT_n + b_hn))
    out = n + z * (h - n)

r/z-gate matmuls from both inputs accumulate into the same PSUM bank, so the
i+h adds are free; per-partition gate biases ride along on the ScalarE
activation (b_hn is folded in as a K=1 ones-row matmul). Matmul operands are
fp16 (fp32 PSUM accumulation, ~1e-4 relative error, half the DMA bytes);
set DT_MODE="f32" for bit-conservative fp32 matmuls. The host then scatters
the updated rows back into a copy of the table with last-occurrence-wins
semantics, matching XLA CPU scatter.
"""

import numpy as np

import concourse.bass as bass
import concourse.mybir as mybir
from concourse import bacc
from concourse.tile import TileContext
from concourse.bass_utils import run_bass_kernel_spmd

F32 = mybir.dt.float32
F16 = mybir.dt.float16
AFT = mybir.ActivationFunctionType

N_CORES = 8
MACRO = 2048  # columns per DMA macro-tile
SUB = 512     # columns per PSUM sub-tile
DT_MODE = "f16"  # "f16" (fast, ~1e-4 rel err) or "f32" (exact, PE-bound)

_NC_CACHE = {}


def build_gru_kernel(n_cols, dt_mode=DT_MODE, macro=MACRO, sub=SUB):
    """Per-core GRU kernel: xT [256,n_cols], hT [128,n_cols] -> outT [128,n_cols]."""
    f16 = dt_mode == "f16"
    MT = F16 if f16 else F32   # matmul operand dtype
    WT = F16 if f16 else F32   # work-tile dtype (fp16 enables DVE 2x modes)
    ODT = F16 if f16 else F32  # output dtype (host upcasts)
    bias_mm = f16  # fold b_hn via K=1 ones-row matmul (cheap at 1 cyc/row)
    nc = bacc.Bacc("TRN2", target_bir_lowering=False, debug=False)
    xT = nc.dram_tensor("xT", [256, n_cols], MT, kind="ExternalInput")
    hT = nc.dram_tensor("hT", [128, n_cols], MT, kind="ExternalInput")
    wihT = nc.dram_tensor("wihT", [256, 384], MT, kind="ExternalInput")
    whhT = nc.dram_tensor("whhT", [128, 384], MT, kind="ExternalInput")
    biasd = nc.dram_tensor("bias", [128, 4], F32, kind="ExternalInput")
    bhnd = nc.dram_tensor("bhn", [1, 128], MT, kind="ExternalInput")
    outT = nc.dram_tensor("outT", [128, n_cols], ODT, kind="ExternalOutput")

    with TileContext(nc) as tc:
        with (
            tc.tile_pool(name="const", bufs=1) as cpool,
            tc.tile_pool(name="io", bufs=2) as iopool,
            tc.tile_pool(name="work", bufs=4) as wpool,
            tc.tile_pool(name="psum", bufs=2, space="PSUM") as ppool,
        ):
            wih0 = cpool.tile([128, 384], MT, tag="wih0")
            wih1 = cpool.tile([128, 384], MT, tag="wih1")
            whh = cpool.tile([128, 384], MT, tag="whh")
            bt = cpool.tile([128, 4], F32, tag="bt")
            nc.sync.dma_start(out=wih0[:], in_=wihT[0:128, :])
            nc.sync.dma_start(out=wih1[:], in_=wihT[128:256, :])
            nc.sync.dma_start(out=whh[:], in_=whhT[:, :])
            nc.sync.dma_start(out=bt[:], in_=biasd[:, :])
            if bias_mm:
                bhn = cpool.tile([1, 128], MT, tag="bhn")
                ones = cpool.tile([1, sub], MT, tag="ones")
                nc.sync.dma_start(out=bhn[:], in_=bhnd[:, :])
                nc.gpsimd.memset(ones[:], 1.0)

            for j in range(n_cols // macro):
                c0 = j * macro
                x0 = iopool.tile([128, macro], MT, tag="x0")
                x1 = iopool.tile([128, macro], MT, tag="x1")
                ht = iopool.tile([128, macro], MT, tag="h")
                ot = iopool.tile([128, macro], ODT, tag="o")
                nc.sync.dma_start(out=x0[:], in_=xT[0:128, c0 : c0 + macro])
                nc.sync.dma_start(out=x1[:], in_=xT[128:256, c0 : c0 + macro])
                nc.sync.dma_start(out=ht[:], in_=hT[:, c0 : c0 + macro])

                for s in range(macro // sub):
                    sl = bass.ts(s, sub)
                    p_r = ppool.tile([128, sub], F32, tag="pr")
                    p_z = ppool.tile([128, sub], F32, tag="pz")
                    p_ni = ppool.tile([128, sub], F32, tag="pni")
                    p_nh = ppool.tile([128, sub], F32, tag="pnh")
                    nc.tensor.matmul(p_r[:], wih0[:, 0:128], x0[:, sl], start=True, stop=False)
                    nc.tensor.matmul(p_r[:], wih1[:, 0:128], x1[:, sl], start=False, stop=False)
                    nc.tensor.matmul(p_r[:], whh[:, 0:128], ht[:, sl], start=False, stop=True)
                    nc.tensor.matmul(p_z[:], wih0[:, 128:256], x0[:, sl], start=True, stop=False)
                    nc.tensor.matmul(p_z[:], wih1[:, 128:256], x1[:, sl], start=False, stop=False)
                    nc.tensor.matmul(p_z[:], whh[:, 128:256], ht[:, sl], start=False, stop=True)
                    nc.tensor.matmul(p_ni[:], wih0[:, 256:384], x0[:, sl], start=True, stop=False)
                    nc.tensor.matmul(p_ni[:], wih1[:, 256:384], x1[:, sl], start=False, stop=True)
                    r = wpool.tile([128, sub], WT, tag="r")
                    z = wpool.tile([128, sub], WT, tag="z")
                    t_ = wpool.tile([128, sub], F32, tag="t")
                    a = wpool.tile([128, sub], F32, tag="a")
                    n = wpool.tile([128, sub], WT, tag="n")
                    s_ = wpool.tile([128, sub], WT, tag="s")
                    m = wpool.tile([128, sub], WT, tag="m")
                    if bias_mm:
                        nc.tensor.matmul(p_nh[:], whh[:, 256:384], ht[:, sl], start=True, stop=False)
                        nc.tensor.matmul(p_nh[:], bhn[:1, :], ones[:1, :], start=False, stop=True)
                        nc.scalar.activation(r[:], p_r[:], AFT.Sigmoid, bias=bt[:, 0:1])
                        nc.scalar.activation(z[:], p_z[:], AFT.Sigmoid, bias=bt[:, 1:2])
                        nc.vector.tensor_mul(t_[:], r[:], p_nh[:])
                    else:
                        nc.tensor.matmul(p_nh[:], whh[:, 256:384], ht[:, sl], start=True, stop=True)
                        hb = wpool.tile([128, sub], F32, tag="hb")
                        nc.scalar.activation(r[:], p_r[:], AFT.Sigmoid, bias=bt[:, 0:1])
                        nc.scalar.activation(z[:], p_z[:], AFT.Sigmoid, bias=bt[:, 1:2])
                        nc.scalar.activation(hb[:], p_nh[:], AFT.Identity, bias=bt[:, 2:3])
                        nc.vector.tensor_mul(t_[:], r[:], hb[:])
                    nc.vector.tensor_add(a[:], t_[:], p_ni[:])
                    nc.scalar.activation(n[:], a[:], AFT.Tanh, bias=bt[:, 3:4])
                    nc.gpsimd.tensor_sub(s_[:], ht[:, sl], n[:])
                    nc.vector.tensor_mul(m[:], z[:], s_[:])
                    nc.vector.tensor_add(ot[:, sl], m[:], n[:])

                nc.sync.dma_start(out=outT[:, c0 : c0 + macro], in_=ot[:])
    nc.finalize()
    return nc


def _host_prep(memory, node_ids, messages, W_ih, W_hh, b_ih, b_hh, dt_mode=DT_MODE):
    """Gather + transpose + pack per-core input maps."""
    mt = np.float16 if dt_mode == "f16" else np.float32
    m = node_ids.shape[0]
    chunk = N_CORES * MACRO
    m_pad = ((m + chunk - 1) // chunk) * chunk

    h = memory[node_ids]  # [m, H] gather on host
    xT = np.zeros((messages.shape[1], m_pad), dtype=mt)
    xT[:, :m] = messages.T.astype(mt)
    hT = np.zeros((memory.shape[1], m_pad), dtype=mt)
    hT[:, :m] = h.T.astype(mt)

    b = (b_ih + b_hh).astype(np.float32)
    bias = np.ascontiguousarray(
        np.stack([b[0:128], b[128:256], b_hh[256:384].astype(np.float32), b_ih[256:384].astype(np.float32)], axis=1),
        dtype=np.float32,
    )
    bhn = np.ascontiguousarray(b_hh[256:384].astype(mt).reshape(1, 128))
    wihT = np.ascontiguousarray(W_ih.T.astype(mt))
    whhT = np.ascontiguousarray(W_hh.T.astype(mt))

    per = m_pad // N_CORES
    in_maps = []
    for c in range(N_CORES):
        sl = slice(c * per, (c + 1) * per)
        in_maps.append(
            {
                "xT": np.ascontiguousarray(xT[:, sl]),
                "hT": np.ascontiguousarray(hT[:, sl]),
                "wihT": wihT,
                "whhT": whhT,
                "bias": bias,
                "bhn": bhn,
            }
        )
    return in_maps, per, m


def kernel(memory, node_ids, messages, W_ih, W_hh, b_ih, b_hh):
    memory = np.ascontiguousarray(np.asarray(memory), dtype=np.float32)
    node_ids = np.asarray(node_ids)
    messages = np.ascontiguousarray(np.asarray(messages), dtype=np.float32)
    W_ih = np.asarray(W_ih, dtype=np.float32)
    W_hh = np.asarray(W_hh, dtype=np.float32)
    b_ih = np.asarray(b_ih, dtype=np.float32)
    b_hh = np.asarray(b_hh, dtype=np.float32)

    in_maps, per, m = _host_prep(memory, node_ids, messages, W_ih, W_hh, b_ih, b_hh)
    key = (per, DT_MODE)
    if key not in _NC_CACHE:
        _NC_CACHE[key] = build_gru_kernel(per)
    nc = _NC_CACHE[key]
    res = None
    for attempt in range(3):
        try:
            res = run_bass_kernel_spmd(nc, in_maps, core_ids=list(range(N_CORES)))
            break
        except Exception:
            if attempt == 2:
                raise
    outT = np.concatenate([r["outT"] for r in res.results], axis=1)
    updated = np.ascontiguousarray(outT[:, :m].T.astype(np.float32))  # [m, H]

    out = memory.copy()
    # scatter, last-occurrence wins (matches XLA CPU .at[].set semantics)
    rev = node_ids[::-1]
    uniq, pos_rev = np.unique(rev, return_index=True)
    out[uniq] = updated[m - 1 - pos_rev]
    return out


# revision 19
# speedup vs baseline: 1.0036x; 1.0036x over previous
"""Trainium2 kernel for nn_NodeMemory (scatter_memory GRU node-memory update).

Strategy
--------
The 512 MB memory table never touches the device. On the host we gather the
131072 referenced rows (memory[node_ids]), transpose the batch into
feature-major layout, and split the update batch evenly across the 8
NeuronCores. Each core runs a Bass/Tile kernel computing the GRU cell for its
16384 rows with the hidden/gate dimension on SBUF partitions:

    giT = W_ih @ x.T   (2 K-tiles of 128)      ghT = W_hh @ h.T   (1 K-tile)
    r = sigmoid(giT_r + ghT_r + b_r)           z = sigmoid(giT_z + ghT_z + b_z)
    n = tanh(giT_n + b_in + r * (ghT_n + b_hn))
    out = n + z * (h - n)

r/z-gate matmuls from both inputs accumulate into the same PSUM bank, so the
i+h adds are free; per-partition gate biases ride along on the ScalarE
activation (b_hn is folded in as a K=1 ones-row matmul). Matmul operands are
fp16 (fp32 PSUM accumulation, ~1e-4 relative error, half the DMA bytes);
set DT_MODE="f32" for bit-conservative fp32 matmuls. The host then scatters
the updated rows back into a copy of the table with last-occurrence-wins
semantics, matching XLA CPU scatter.
"""

import numpy as np

import concourse.bass as bass
import concourse.mybir as mybir
from concourse import bacc
from concourse.tile import TileContext
from concourse.bass_utils import run_bass_kernel_spmd

F32 = mybir.dt.float32
F16 = mybir.dt.float16
AFT = mybir.ActivationFunctionType

N_CORES = 8
MACRO = 2048  # columns per DMA macro-tile
SUB = 512     # columns per PSUM sub-tile
DT_MODE = "f16"  # "f16" (fast, ~1e-4 rel err) or "f32" (exact, PE-bound)

_NC_CACHE = {}


def build_gru_kernel(n_cols, dt_mode=DT_MODE, macro=MACRO, sub=SUB):
    """Per-core GRU kernel: xT [256,n_cols], hT [128,n_cols] -> outT [128,n_cols]."""
    f16 = dt_mode == "f16"
    MT = F16 if f16 else F32   # matmul operand dtype
    WT = F16 if f16 else F32   # work-tile dtype (fp16 enables DVE 2x modes)
    ODT = F16 if f16 else F32  # output dtype (host upcasts)
    bias_mm = f16  # fold b_hn via K=1 ones-row matmul (cheap at 1 cyc/row)
    nc = bacc.Bacc("TRN2", target_bir_lowering=False, debug=False)
    xT = nc.dram_tensor("xT", [256, n_cols], MT, kind="ExternalInput")
    hT = nc.dram_tensor("hT", [128, n_cols], MT, kind="ExternalInput")
    wihT = nc.dram_tensor("wihT", [256, 384], MT, kind="ExternalInput")
    whhT = nc.dram_tensor("whhT", [128, 384], MT, kind="ExternalInput")
    biasd = nc.dram_tensor("bias", [128, 4], F32, kind="ExternalInput")
    bhnd = nc.dram_tensor("bhn", [1, 128], MT, kind="ExternalInput")
    outT = nc.dram_tensor("outT", [128, n_cols], ODT, kind="ExternalOutput")

    with TileContext(nc) as tc:
        with (
            tc.tile_pool(name="const", bufs=1) as cpool,
            tc.tile_pool(name="io", bufs=2) as iopool,
            tc.tile_pool(name="mac", bufs=2) as mpool,
            tc.tile_pool(name="work", bufs=4) as wpool,
            tc.tile_pool(name="psum", bufs=2, space="PSUM") as ppool,
        ):
            wih0 = cpool.tile([128, 384], MT, tag="wih0")
            wih1 = cpool.tile([128, 384], MT, tag="wih1")
            whh = cpool.tile([128, 384], MT, tag="whh")
            bt = cpool.tile([128, 4], F32, tag="bt")
            nc.sync.dma_start(out=wih0[:], in_=wihT[0:128, :])
            nc.sync.dma_start(out=wih1[:], in_=wihT[128:256, :])
            nc.sync.dma_start(out=whh[:], in_=whhT[:, :])
            nc.sync.dma_start(out=bt[:], in_=biasd[:, :])
            if bias_mm:
                bhn = cpool.tile([1, 128], MT, tag="bhn")
                ones = cpool.tile([1, sub], MT, tag="ones")
                nc.sync.dma_start(out=bhn[:], in_=bhnd[:, :])
                nc.vector.memset(ones[:], 1.0)

            for j in range(n_cols // macro):
                c0 = j * macro
                x0 = iopool.tile([128, macro], MT, tag="x0")
                x1 = iopool.tile([128, macro], MT, tag="x1")
                ht = iopool.tile([128, macro], MT, tag="h")
                if j == 0:
                    # split the first macro's loads per subtile so the PE can
                    # start after ~0.4 MB instead of waiting for the full tile
                    for q in range(macro // sub):
                        qs = bass.ts(q, sub)
                        nc.sync.dma_start(out=x0[:, qs], in_=xT[0:128, c0 + q * sub : c0 + (q + 1) * sub])
                        nc.sync.dma_start(out=x1[:, qs], in_=xT[128:256, c0 + q * sub : c0 + (q + 1) * sub])
                        nc.sync.dma_start(out=ht[:, qs], in_=hT[:, c0 + q * sub : c0 + (q + 1) * sub])
                else:
                    nc.sync.dma_start(out=x0[:], in_=xT[0:128, c0 : c0 + macro])
                    nc.sync.dma_start(out=x1[:], in_=xT[128:256, c0 : c0 + macro])
                    nc.sync.dma_start(out=ht[:], in_=hT[:, c0 : c0 + macro])
                if f16:
                    # n and m land in macro-wide buffers; the output is
                    # assembled in DRAM as outT = n, outT += m via two SWDGE
                    # DMAs (same queue, FIFO) — no DVE/Pool combine op.
                    nmac = mpool.tile([128, macro], ODT, tag="nm")
                    mmac = mpool.tile([128, macro], ODT, tag="mm")
                else:
                    ot = mpool.tile([128, macro], ODT, tag="ot")

                for s in range(macro // sub):
                    sl = bass.ts(s, sub)
                    p_r = ppool.tile([128, sub], F32, tag="pr")
                    p_z = ppool.tile([128, sub], F32, tag="pz")
                    p_ni = ppool.tile([128, sub], F32, tag="pni")
                    p_nh = ppool.tile([128, sub], F32, tag="pnh")
                    nc.tensor.matmul(p_r[:], wih0[:, 0:128], x0[:, sl], start=True, stop=False)
                    nc.tensor.matmul(p_r[:], wih1[:, 0:128], x1[:, sl], start=False, stop=False)
                    nc.tensor.matmul(p_r[:], whh[:, 0:128], ht[:, sl], start=False, stop=True)
                    nc.tensor.matmul(p_z[:], wih0[:, 128:256], x0[:, sl], start=True, stop=False)
                    nc.tensor.matmul(p_z[:], wih1[:, 128:256], x1[:, sl], start=False, stop=False)
                    nc.tensor.matmul(p_z[:], whh[:, 128:256], ht[:, sl], start=False, stop=True)
                    nc.tensor.matmul(p_ni[:], wih0[:, 256:384], x0[:, sl], start=True, stop=False)
                    nc.tensor.matmul(p_ni[:], wih1[:, 256:384], x1[:, sl], start=False, stop=True)
                    r = wpool.tile([128, sub], WT, tag="r")
                    z = wpool.tile([128, sub], WT, tag="z")
                    t_ = wpool.tile([128, sub], F32, tag="t")
                    a = wpool.tile([128, sub], F32, tag="a")
                    s_ = wpool.tile([128, sub], WT, tag="s")
                    if bias_mm:
                        nc.tensor.matmul(p_nh[:], whh[:, 256:384], ht[:, sl], start=True, stop=False)
                        nc.tensor.matmul(p_nh[:], bhn[:1, :], ones[:1, :], start=False, stop=True)
                        nc.scalar.activation(r[:], p_r[:], AFT.Sigmoid, bias=bt[:, 0:1])
                        nc.scalar.activation(z[:], p_z[:], AFT.Sigmoid, bias=bt[:, 1:2])
                        nc.vector.tensor_mul(t_[:], r[:], p_nh[:])
                    else:
                        nc.tensor.matmul(p_nh[:], whh[:, 256:384], ht[:, sl], start=True, stop=True)
                        hb = wpool.tile([128, sub], F32, tag="hb")
                        nc.scalar.activation(r[:], p_r[:], AFT.Sigmoid, bias=bt[:, 0:1])
                        nc.scalar.activation(z[:], p_z[:], AFT.Sigmoid, bias=bt[:, 1:2])
                        nc.scalar.activation(hb[:], p_nh[:], AFT.Identity, bias=bt[:, 2:3])
                        nc.vector.tensor_mul(t_[:], r[:], hb[:])
                    nc.vector.tensor_add(a[:], t_[:], p_ni[:])
                    if f16:
                        nc.scalar.activation(nmac[:, sl], a[:], AFT.Tanh, bias=bt[:, 3:4])
                        nc.vector.tensor_sub(s_[:], ht[:, sl], nmac[:, sl])
                        nc.vector.tensor_mul(mmac[:, sl], z[:], s_[:])
                    else:
                        n = wpool.tile([128, sub], WT, tag="n")
                        m = wpool.tile([128, sub], WT, tag="m")
                        nc.scalar.activation(n[:], a[:], AFT.Tanh, bias=bt[:, 3:4])
                        nc.gpsimd.tensor_sub(s_[:], ht[:, sl], n[:])
                        nc.vector.tensor_mul(m[:], z[:], s_[:])
                        nc.vector.tensor_add(ot[:, sl], m[:], n[:])

                if f16:
                    nc.gpsimd.dma_start(out=outT[:, c0 : c0 + macro], in_=nmac[:])
                    nc.gpsimd.dma_start(out=outT[:, c0 : c0 + macro], in_=mmac[:], accum_op=mybir.AluOpType.add)
                else:
                    nc.sync.dma_start(out=outT[:, c0 : c0 + macro], in_=ot[:])
    nc.finalize()
    return nc


def _host_prep(memory, node_ids, messages, W_ih, W_hh, b_ih, b_hh, dt_mode=DT_MODE):
    """Gather + transpose + pack per-core input maps."""
    mt = np.float16 if dt_mode == "f16" else np.float32
    m = node_ids.shape[0]
    chunk = N_CORES * MACRO
    m_pad = ((m + chunk - 1) // chunk) * chunk

    h = memory[node_ids]  # [m, H] gather on host
    xT = np.zeros((messages.shape[1], m_pad), dtype=mt)
    xT[:, :m] = messages.T.astype(mt)
    hT = np.zeros((memory.shape[1], m_pad), dtype=mt)
    hT[:, :m] = h.T.astype(mt)

    b = (b_ih + b_hh).astype(np.float32)
    bias = np.ascontiguousarray(
        np.stack([b[0:128], b[128:256], b_hh[256:384].astype(np.float32), b_ih[256:384].astype(np.float32)], axis=1),
        dtype=np.float32,
    )
    bhn = np.ascontiguousarray(b_hh[256:384].astype(mt).reshape(1, 128))
    wihT = np.ascontiguousarray(W_ih.T.astype(mt))
    whhT = np.ascontiguousarray(W_hh.T.astype(mt))

    per = m_pad // N_CORES
    in_maps = []
    for c in range(N_CORES):
        sl = slice(c * per, (c + 1) * per)
        in_maps.append(
            {
                "xT": np.ascontiguousarray(xT[:, sl]),
                "hT": np.ascontiguousarray(hT[:, sl]),
                "wihT": wihT,
                "whhT": whhT,
                "bias": bias,
                "bhn": bhn,
            }
        )
    return in_maps, per, m


def kernel(memory, node_ids, messages, W_ih, W_hh, b_ih, b_hh):
    memory = np.ascontiguousarray(np.asarray(memory), dtype=np.float32)
    node_ids = np.asarray(node_ids)
    messages = np.ascontiguousarray(np.asarray(messages), dtype=np.float32)
    W_ih = np.asarray(W_ih, dtype=np.float32)
    W_hh = np.asarray(W_hh, dtype=np.float32)
    b_ih = np.asarray(b_ih, dtype=np.float32)
    b_hh = np.asarray(b_hh, dtype=np.float32)

    in_maps, per, m = _host_prep(memory, node_ids, messages, W_ih, W_hh, b_ih, b_hh)
    key = (per, DT_MODE)
    if key not in _NC_CACHE:
        _NC_CACHE[key] = build_gru_kernel(per)
    nc = _NC_CACHE[key]
    res = None
    for attempt in range(3):
        try:
            res = run_bass_kernel_spmd(nc, in_maps, core_ids=list(range(N_CORES)))
            break
        except Exception:
            if attempt == 2:
                raise
    outT = np.concatenate([r["outT"] for r in res.results], axis=1)
    updated = np.ascontiguousarray(outT[:, :m].T.astype(np.float32))  # [m, H]

    out = memory.copy()
    # scatter, last-occurrence wins (matches XLA CPU .at[].set semantics)
    rev = node_ids[::-1]
    uniq, pos_rev = np.unique(rev, return_index=True)
    out[uniq] = updated[m - 1 - pos_rev]
    return out


# revision 21
# speedup vs baseline: 1.2949x; 1.2902x over previous
"""Trainium2 kernel for nn_NodeMemory (scatter_memory GRU node-memory update).

Strategy
--------
The 512 MB memory table never touches the device. On the host we gather the
131072 referenced rows (memory[node_ids]), transpose the batch into
feature-major layout, and split the update batch evenly across the 8
NeuronCores. Each core runs a Bass/Tile kernel computing the GRU cell for its
16384 rows with the hidden/gate dimension on SBUF partitions:

    giT = W_ih @ x.T   (2 K-tiles of 128)      ghT = W_hh @ h.T   (1 K-tile)
    r = sigmoid(giT_r + ghT_r + b_r)           z = sigmoid(giT_z + ghT_z + b_z)
    n = tanh(giT_n + b_in + r * (ghT_n + b_hn))
    out = n + z * (h - n)

r/z-gate matmuls from both inputs accumulate into the same PSUM bank, so the
i+h adds are free; per-partition gate biases ride along on the ScalarE
activation (b_hn is folded in as a K=1 ones-row matmul). Matmul operands are
fp16 (fp32 PSUM accumulation, ~1e-4 relative error, half the DMA bytes);
set DT_MODE="f32" for bit-conservative fp32 matmuls. The host then scatters
the updated rows back into a copy of the table with last-occurrence-wins
semantics, matching XLA CPU scatter.
"""

import numpy as np

import concourse.bass as bass
import concourse.mybir as mybir
from concourse import bacc
from concourse.tile import TileContext
from concourse.bass_utils import run_bass_kernel_spmd

F32 = mybir.dt.float32
F16 = mybir.dt.float16
AFT = mybir.ActivationFunctionType

N_CORES = 8
MACRO = 2048  # columns per DMA macro-tile
SUB = 512     # columns per PSUM sub-tile
DT_MODE = "f16"  # "f16" (fast, ~1e-4 rel err) or "f32" (exact, PE-bound)

_NC_CACHE = {}


def build_gru_kernel(n_cols, dt_mode=DT_MODE, macro=MACRO, sub=SUB):
    """Per-core GRU kernel: xT [256,n_cols], hT [128,n_cols] -> outT [128,n_cols]."""
    f16 = dt_mode == "f16"
    MT = F16 if f16 else F32   # matmul operand dtype
    WT = F16 if f16 else F32   # work-tile dtype (fp16 enables DVE 2x modes)
    ODT = F16 if f16 else F32  # output dtype (host upcasts)
    bias_mm = f16  # fold b_hn via K=1 ones-row matmul (cheap at 1 cyc/row)
    nc = bacc.Bacc("TRN2", target_bir_lowering=False, debug=False)
    xT = nc.dram_tensor("xT", [256, n_cols], MT, kind="ExternalInput")
    hT = nc.dram_tensor("hT", [128, n_cols], MT, kind="ExternalInput")
    wihT = nc.dram_tensor("wihT", [256, 384], MT, kind="ExternalInput")
    whhT = nc.dram_tensor("whhT", [128, 384], MT, kind="ExternalInput")
    biasd = nc.dram_tensor("bias", [128, 4], F32, kind="ExternalInput")
    bhnd = nc.dram_tensor("bhn", [1, 128], MT, kind="ExternalInput")
    outT = nc.dram_tensor("outT", [128, n_cols], ODT, kind="ExternalOutput")

    with TileContext(nc) as tc:
        with (
            tc.tile_pool(name="const", bufs=1) as cpool,
            tc.tile_pool(name="io", bufs=2) as iopool,
            tc.tile_pool(name="mac", bufs=2) as mpool,
            tc.tile_pool(name="work", bufs=4) as wpool,
            tc.tile_pool(name="psum", bufs=2, space="PSUM") as ppool,
        ):
            wih0 = cpool.tile([128, 384], MT, tag="wih0")
            wih1 = cpool.tile([128, 384], MT, tag="wih1")
            whh = cpool.tile([128, 384], MT, tag="whh")
            bt = cpool.tile([128, 4], F32, tag="bt")
            nc.sync.dma_start(out=wih0[:], in_=wihT[0:128, :])
            nc.sync.dma_start(out=wih1[:], in_=wihT[128:256, :])
            nc.sync.dma_start(out=whh[:], in_=whhT[:, :])
            nc.sync.dma_start(out=bt[:], in_=biasd[:, :])
            if bias_mm:
                bhn = cpool.tile([1, 128], MT, tag="bhn")
                ones = cpool.tile([1, sub], MT, tag="ones")
                nc.sync.dma_start(out=bhn[:], in_=bhnd[:, :])
                nc.vector.memset(ones[:], 1.0)

            for j in range(n_cols // macro):
                c0 = j * macro
                x0 = iopool.tile([128, macro], MT, tag="x0")
                x1 = iopool.tile([128, macro], MT, tag="x1")
                ht = iopool.tile([128, macro], MT, tag="h")
                if j == 0:
                    # split the first macro's loads per subtile so the PE can
                    # start after ~0.4 MB instead of waiting for the full tile
                    for q in range(macro // sub):
                        qs = bass.ts(q, sub)
                        nc.sync.dma_start(out=x0[:, qs], in_=xT[0:128, c0 + q * sub : c0 + (q + 1) * sub])
                        nc.sync.dma_start(out=x1[:, qs], in_=xT[128:256, c0 + q * sub : c0 + (q + 1) * sub])
                        nc.sync.dma_start(out=ht[:, qs], in_=hT[:, c0 + q * sub : c0 + (q + 1) * sub])
                else:
                    nc.sync.dma_start(out=x0[:], in_=xT[0:128, c0 : c0 + macro])
                    nc.sync.dma_start(out=x1[:], in_=xT[128:256, c0 : c0 + macro])
                    nc.sync.dma_start(out=ht[:], in_=hT[:, c0 : c0 + macro])
                if f16:
                    # n and m land in macro-wide buffers; the output is
                    # assembled in DRAM as outT = n, outT += m via two SWDGE
                    # DMAs (same queue, FIFO) — no DVE/Pool combine op.
                    nmac = mpool.tile([128, macro], ODT, tag="nm")
                    mmac = mpool.tile([128, macro], ODT, tag="mm")
                else:
                    ot = mpool.tile([128, macro], ODT, tag="ot")

                for s in range(macro // sub):
                    sl = bass.ts(s, sub)
                    p_r = ppool.tile([128, sub], F32, tag="pr")
                    p_z = ppool.tile([128, sub], F32, tag="pz")
                    p_ni = ppool.tile([128, sub], F32, tag="pni")
                    p_nh = ppool.tile([128, sub], F32, tag="pnh")
                    nc.tensor.matmul(p_r[:], wih0[:, 0:128], x0[:, sl], start=True, stop=False)
                    nc.tensor.matmul(p_r[:], wih1[:, 0:128], x1[:, sl], start=False, stop=False)
                    nc.tensor.matmul(p_r[:], whh[:, 0:128], ht[:, sl], start=False, stop=True)
                    nc.tensor.matmul(p_z[:], wih0[:, 128:256], x0[:, sl], start=True, stop=False)
                    nc.tensor.matmul(p_z[:], wih1[:, 128:256], x1[:, sl], start=False, stop=False)
                    nc.tensor.matmul(p_z[:], whh[:, 128:256], ht[:, sl], start=False, stop=True)
                    nc.tensor.matmul(p_ni[:], wih0[:, 256:384], x0[:, sl], start=True, stop=False)
                    nc.tensor.matmul(p_ni[:], wih1[:, 256:384], x1[:, sl], start=False, stop=True)
                    r = wpool.tile([128, sub], WT, tag="r")
                    z = wpool.tile([128, sub], WT, tag="z")
                    t_ = wpool.tile([128, sub], F32, tag="t")
                    a = wpool.tile([128, sub], F32, tag="a")
                    s_ = wpool.tile([128, sub], WT, tag="s")
                    if bias_mm:
                        nc.tensor.matmul(p_nh[:], whh[:, 256:384], ht[:, sl], start=True, stop=False)
                        nc.tensor.matmul(p_nh[:], bhn[:1, :], ones[:1, :], start=False, stop=True)
                        nc.scalar.activation(r[:], p_r[:], AFT.Sigmoid, bias=bt[:, 0:1])
                        nc.scalar.activation(z[:], p_z[:], AFT.Sigmoid, bias=bt[:, 1:2])
                        nc.vector.tensor_mul(t_[:], r[:], p_nh[:])
                    else:
                        nc.tensor.matmul(p_nh[:], whh[:, 256:384], ht[:, sl], start=True, stop=True)
                        hb = wpool.tile([128, sub], F32, tag="hb")
                        nc.scalar.activation(r[:], p_r[:], AFT.Sigmoid, bias=bt[:, 0:1])
                        nc.scalar.activation(z[:], p_z[:], AFT.Sigmoid, bias=bt[:, 1:2])
                        nc.scalar.activation(hb[:], p_nh[:], AFT.Identity, bias=bt[:, 2:3])
                        nc.vector.tensor_mul(t_[:], r[:], hb[:])
                    nc.vector.tensor_add(a[:], t_[:], p_ni[:])
                    if f16:
                        nc.scalar.activation(nmac[:, sl], a[:], AFT.Tanh, bias=bt[:, 3:4])
                        nc.vector.tensor_sub(s_[:], ht[:, sl], nmac[:, sl])
                        nc.vector.tensor_mul(mmac[:, sl], z[:], s_[:])
                    else:
                        n = wpool.tile([128, sub], WT, tag="n")
                        m = wpool.tile([128, sub], WT, tag="m")
                        nc.scalar.activation(n[:], a[:], AFT.Tanh, bias=bt[:, 3:4])
                        nc.gpsimd.tensor_sub(s_[:], ht[:, sl], n[:])
                        nc.vector.tensor_mul(m[:], z[:], s_[:])
                        nc.vector.tensor_add(ot[:, sl], m[:], n[:])

                if f16:
                    nc.gpsimd.dma_start(out=outT[:, c0 : c0 + macro], in_=nmac[:])
                    nc.gpsimd.dma_start(out=outT[:, c0 : c0 + macro], in_=mmac[:], accum_op=mybir.AluOpType.add)
                else:
                    nc.sync.dma_start(out=outT[:, c0 : c0 + macro], in_=ot[:])
    nc.finalize()
    return nc


def _host_prep(memory, node_ids, messages, W_ih, W_hh, b_ih, b_hh, dt_mode=DT_MODE):
    """Gather + transpose + pack per-core input maps."""
    mt = np.float16 if dt_mode == "f16" else np.float32
    m = node_ids.shape[0]
    chunk = N_CORES * MACRO
    m_pad = ((m + chunk - 1) // chunk) * chunk

    h = memory[node_ids]  # [m, H] gather on host
    xT = np.zeros((messages.shape[1], m_pad), dtype=mt)
    xT[:, :m] = messages.T.astype(mt)
    hT = np.zeros((memory.shape[1], m_pad), dtype=mt)
    hT[:, :m] = h.T.astype(mt)

    b = (b_ih + b_hh).astype(np.float32)
    bias = np.ascontiguousarray(
        np.stack([b[0:128], b[128:256], b_hh[256:384].astype(np.float32), b_ih[256:384].astype(np.float32)], axis=1),
        dtype=np.float32,
    )
    bhn = np.ascontiguousarray(b_hh[256:384].astype(mt).reshape(1, 128))
    wihT = np.ascontiguousarray(W_ih.T.astype(mt))
    whhT = np.ascontiguousarray(W_hh.T.astype(mt))

    per = m_pad // N_CORES
    in_maps = []
    for c in range(N_CORES):
        sl = slice(c * per, (c + 1) * per)
        in_maps.append(
            {
                "xT": np.ascontiguousarray(xT[:, sl]),
                "hT": np.ascontiguousarray(hT[:, sl]),
                "wihT": wihT,
                "whhT": whhT,
                "bias": bias,
                "bhn": bhn,
            }
        )
    return in_maps, per, m


def _exec_once(per, in_maps):
    key = (per, DT_MODE)
    if key not in _NC_CACHE:
        _NC_CACHE[key] = build_gru_kernel(per)
    res = run_bass_kernel_spmd(_NC_CACHE[key], in_maps, core_ids=list(range(N_CORES)))
    return np.concatenate([r["outT"] for r in res.results], axis=1)


def _exec_subprocess(per, in_maps):
    """Last-resort retry in a fresh process — a transient NRT device failure
    poisons the in-process PJRT backend, but a new process re-boots cleanly."""
    import os, subprocess, sys, tempfile

    mydir = os.path.dirname(os.path.abspath(__file__))
    with tempfile.TemporaryDirectory() as td:
        inp, outp = os.path.join(td, "in.npz"), os.path.join(td, "out.npy")
        np.savez(inp, **{f"{k}_{c}": v for c, im in enumerate(in_maps) for k, v in im.items()})
        child = (
            "import sys, numpy as np\n"
            f"sys.path.insert(0, {mydir!r})\n"
            "import kernel as K\n"
            f"d = np.load({inp!r})\n"
            f"n = {len(in_maps)}\n"
            "ims = [{k[: k.rfind('_')]: d[k] for k in d.files if int(k[k.rfind('_')+1:]) == c} for c in range(n)]\n"
            f"np.save({outp!r}, K._exec_once({per}, ims))\n"
        )
        subprocess.run([sys.executable, "-c", child], check=True, timeout=1800)
        return np.load(outp)


def _run_device(per, in_maps):
    try:
        return _exec_once(per, in_maps)
    except Exception:
        pass
    try:  # drop the (possibly poisoned) PJRT backend and retry in-process
        import jax
        from jax.extend import backend as _jeb

        _jeb.clear_backends()
        jax.clear_caches()
        return _exec_once(per, in_maps)
    except Exception:
        pass
    return _exec_subprocess(per, in_maps)


def kernel(memory, node_ids, messages, W_ih, W_hh, b_ih, b_hh):
    memory = np.ascontiguousarray(np.asarray(memory), dtype=np.float32)
    node_ids = np.asarray(node_ids)
    messages = np.ascontiguousarray(np.asarray(messages), dtype=np.float32)
    W_ih = np.asarray(W_ih, dtype=np.float32)
    W_hh = np.asarray(W_hh, dtype=np.float32)
    b_ih = np.asarray(b_ih, dtype=np.float32)
    b_hh = np.asarray(b_hh, dtype=np.float32)

    in_maps, per, m = _host_prep(memory, node_ids, messages, W_ih, W_hh, b_ih, b_hh)
    outT = _run_device(per, in_maps)
    updated = np.ascontiguousarray(outT[:, :m].T.astype(np.float32))  # [m, H]

    out = memory.copy()
    # scatter, last-occurrence wins (matches XLA CPU .at[].set semantics)
    rev = node_ids[::-1]
    uniq, pos_rev = np.unique(rev, return_index=True)
    out[uniq] = updated[m - 1 - pos_rev]
    return out
